# revision 1
# baseline (speedup 1.0000x reference)
"""Trainium2 Bass kernel for nn_MetaNetLinearizedModel (optimized).

Same math as v2 but with a restructured post-AllReduce tail:
    out_c = csel_c * (z1 @ W2 + g @ dW2[c]) + [c==0] * (g @ W2)
    z1 = gp * (U_c/32 + db1[c])       (csel row-scaling commutes past @W2)
The tail runs in transposed (f-major) layout: h1r is transposed once,
gelu/gelu' evaluate directly into [f, b] stationaries, and the final
combine is 2 vector ops per 512/256 half. V's matmuls accumulate into
the same PSUM group as z1@W2. Activation LUTs are preloaded at t~0.
The fp8 dW1 stream is split across both HWDGE rings (sync+scalar, each
caps ~150 GB/s).
"""
import sys

sys.path.insert(0, "/opt/trn_rl_repo")

import numpy as np
import ml_dtypes
import concourse.bass as bass
import concourse.bacc as bacc
import concourse.tile as tile
import concourse.mybir as mybir
from concourse import bass_utils

F32 = mybir.dt.float32
BF16 = mybir.dt.bfloat16
FP8 = mybir.dt.float8e4
AF = mybir.ActivationFunctionType
OP = mybir.AluOpType

B = 16
D = 3 * 64 * 64        # 12288
F = 768
HID = 192
T = 8
NCORES = 8
DSH = D // NCORES      # 1536
KD = D // 128          # 96
KSH = DSH // 128       # 12
KF = F // 128          # 6
FO = F // 128          # 6
FP8_SCALE = 32.0

BC_B1 = 0
BC_DB1 = F
BC_A1V = 2 * F
BC_MB2 = BC_A1V + HID
BC_SEL = BC_MB2 + T
BC_ISB = BC_SEL + T
BC_EYE = BC_ISB + 1
BC_W = BC_EYE + 32

NDW1C = 12             # dW1 chunks (8 k-tiles each), split even/odd by ring

_CACHE = {}


def build():
    nc = bacc.Bacc("TRN2", target_bir_lowering=False, debug=False,
                   enable_asserts=False, num_devices=NCORES)

    XTS = nc.dram_tensor("xts", [128, KSH * B], BF16, kind="ExternalInput")
    XT = nc.dram_tensor("xt", [128, KD * B], BF16, kind="ExternalInput")
    W1S = nc.dram_tensor("w1s", [128, KSH * F], BF16, kind="ExternalInput")
    DW1 = nc.dram_tensor("dw1", [128, KD * F], FP8, kind="ExternalInput")
    W2P = nc.dram_tensor("w2p", [128, KF * F], BF16, kind="ExternalInput")
    DW2P = nc.dram_tensor("dw2p", [128, KF * F], BF16, kind="ExternalInput")
    A1P = nc.dram_tensor("a1p", [128, KF * HID], BF16, kind="ExternalInput")
    MW2T = nc.dram_tensor("mw2t", [128, 2 * T], BF16, kind="ExternalInput")
    BC = nc.dram_tensor("bc", [32, BC_W], F32, kind="ExternalInput")
    FS = nc.dram_tensor("fs", [128, B], F32, kind="ExternalInput")
    # rows 0:16 = g@W2, 32:48 = g@dW2, 64:80 = z1@W2 (16:32, 48:64 junk);
    # the host applies csel / core-0 selection and sums across cores.
    OT = nc.dram_tensor("ot", [80, F], F32, kind="ExternalOutput")
    CS = nc.dram_tensor("cs", [B, 1], F32, kind="ExternalOutput")

    with tile.TileContext(nc, num_cores=NCORES) as tc:
        with (
            tc.tile_pool(name="cst", bufs=1) as cst,
            tc.tile_pool(name="w1c", bufs=3) as w1cp,
            tc.tile_pool(name="dw1p", bufs=NDW1C) as dw1p,
            tc.tile_pool(name="wrk", bufs=1) as wrk,
            tc.tile_pool(name="psu", bufs=1, space="PSUM") as psu,
            tc.tile_pool(name="pss", bufs=3, space="PSUM") as pss,
            tc.tile_pool(name="drm", bufs=1, space="DRAM") as drm,
        ):
            # ---- activation LUT preload (hides the 2x1.3us table loads) ----
            scr = wrk.tile([1, 2], F32)
            nc.vector.memset(scr[:], 0.0)
            scr2 = wrk.tile([1, 2], F32)
            nc.scalar.activation(scr2[:, 0:1], scr[:, 0:1], AF.Gelu_apprx_tanh)

            # ---- DMA kicks; dW1 split even/odd across the two HWDGE rings --
            xts_sb = cst.tile([128, KSH * B], BF16)
            nc.sync.dma_start(xts_sb[:], XTS.ap())
            w1c = []
            for j in range(3):
                wt = w1cp.tile([128, 4 * F], BF16, name="w1t", tag="w1t")
                eng = nc.scalar if j == 2 else nc.sync
                eng.dma_start(wt[:], W1S.ap()[:, j * 4 * F:(j + 1) * 4 * F])
                w1c.append(wt)
            bc_sb = cst.tile([32, BC_W], F32)
            nc.scalar.dma_start(bc_sb[:], BC.ap())
            fs_sb = cst.tile([128, B], F32)
            nc.scalar.dma_start(fs_sb[:], FS.ap())
            xt_sb = cst.tile([128, KD * B], BF16)
            nc.scalar.dma_start(xt_sb[:], XT.ap())
            dchunks = []
            for j in range(NDW1C):
                dt_ = dw1p.tile([128, 8 * F], FP8, name="dw1t", tag="dw1t")
                eng = nc.sync if j % 2 == 0 else nc.scalar
                eng.dma_start(dt_[:], DW1.ap()[:, j * 8 * F:(j + 1) * 8 * F])
                dchunks.append(dt_)
            w2_sb = cst.tile([128, KF * F], BF16)
            nc.sync.dma_start(w2_sb[:], W2P.ap())
            dw2_sb = cst.tile([128, KF * F], BF16)
            nc.scalar.dma_start(dw2_sb[:], DW2P.ap())
            a1_sb = cst.tile([128, KF * HID], BF16)
            nc.sync.dma_start(a1_sb[:], A1P.ap())
            mw2t_sb = cst.tile([128, 2 * T], BF16)
            nc.scalar.dma_start(mw2t_sb[:], MW2T.ap())

            def bcs(col, w):
                return bc_sb[0:B, col:col + w]
            eye = bc_sb[0:32, BC_EYE:BC_EYE + 32]
            isb = bc_sb[0:B, BC_ISB:BC_ISB + 1]
            eyeb = wrk.tile([32, 32], BF16)
            nc.vector.tensor_copy(eyeb[:], eye)
            epsp = wrk.tile([128, 1], F32)
            nc.vector.memset(epsp[:], 0.125)
            epsm = wrk.tile([128, 1], F32)
            nc.vector.memset(epsm[:], -0.125)

            # ---- h1 partial -> AllReduce (bf16 payload) ----
            h5 = pss.tile([B, 512], F32, name="ps", tag="ps")
            h2 = pss.tile([B, 256], F32, name="ps", tag="ps")
            for k in range(KSH):
                wt = w1c[k // 4]
                q = (k % 4) * F
                nc.tensor.matmul(h5[:], xts_sb[:, k * B:(k + 1) * B],
                                 wt[:, q:q + 512],
                                 start=(k == 0), stop=(k == KSH - 1))
                nc.tensor.matmul(h2[:], xts_sb[:, k * B:(k + 1) * B],
                                 wt[:, q + 512:q + F],
                                 start=(k == 0), stop=(k == KSH - 1))
            bI = wrk.tile([B, F], F32)
            nc.vector.tensor_scalar(bI[:], bcs(BC_B1, F), isb, None, OP.mult)
            h1p_sb = wrk.tile([32, F], BF16)
            nc.vector.memset(h1p_sb[:], 0.0)
            nc.vector.tensor_add(h1p_sb[0:B, 0:512], h5[:], bI[:, 0:512])
            nc.vector.tensor_add(h1p_sb[0:B, 512:F], h2[:], bI[:, 512:F])
            # transpose the partial to [f, b] BEFORE the AllReduce (the
            # elementwise sum is layout-free) so the readback lands directly
            # in the f-major layout the tail consumes; PE transposes here
            # cost ~1.6us of DMA-paced (idle) PE time
            h1pt_sb = wrk.tile([128, FO * B], BF16)
            for fo in range(FO):
                tpb = pss.tile([128, 32], BF16, name="ps", tag="ps")
                nc.tensor.transpose(tpb[:],
                                    h1p_sb[0:32, fo * 128:(fo + 1) * 128],
                                    eyeb[:])
                nc.vector.tensor_copy(h1pt_sb[:, fo * B:(fo + 1) * B],
                                      tpb[:, 0:B])
            h1pT_d = drm.tile([128, FO * B], BF16)
            h1rT_d = drm.tile([128, FO * B], BF16)
            nc.gpsimd.dma_start(h1pT_d[:], h1pt_sb[:])
            nc.gpsimd.collective_compute(
                "AllReduce", OP.add,
                replica_groups=[list(range(NCORES))],
                ins=[h1pT_d[:]], outs=[h1rT_d[:]])
            h1rt = wrk.tile([128, FO * B], BF16)
            nc.sync.dma_start(h1rt[:, 0:3 * B], h1rT_d[:, 0:3 * B])
            nc.scalar.dma_start(h1rt[:, 3 * B:FO * B], h1rT_d[:, 3 * B:FO * B])

            # ---- U = X @ dW1 (fp8 moving, 4 PE column groups) ----
            u5 = psu.tile([128, 512], F32, name="u5")
            u2 = psu.tile([128, 256], F32, name="u2")
            for i in range(KD // 4):
                for cg in range(4):
                    k = 4 * i + cg
                    src = dchunks[k // 8]
                    q = (k % 8) * F
                    nc.tensor.matmul(
                        u5[32 * cg:32 * cg + B, :],
                        xt_sb[:, k * B:(k + 1) * B],
                        src[:, q:q + 512],
                        start=(i == 0), stop=(i == KD // 4 - 1),
                        tile_position=(0, 32 * cg),
                        skip_group_check=True)
                    nc.tensor.matmul(
                        u2[32 * cg:32 * cg + B, :],
                        xt_sb[:, k * B:(k + 1) * B],
                        src[:, q + 512:q + F],
                        start=(i == 0), stop=(i == KD // 4 - 1),
                        tile_position=(0, 32 * cg),
                        skip_group_check=True)
            u5_sb = wrk.tile([128, 512], F32)
            u2_sb = wrk.tile([128, 256], F32)
            nc.vector.tensor_copy(u5_sb[:], u5[:])
            nc.vector.tensor_copy(u2_sb[:], u2[:])
            uf5 = pss.tile([B, 512], F32, name="ps", tag="ps")
            uf2 = pss.tile([B, 256], F32, name="ps", tag="ps")
            nc.tensor.matmul(uf5[:], fs_sb[:], u5_sb[:], start=True, stop=True)
            nc.tensor.matmul(uf2[:], fs_sb[:], u2_sb[:], start=True, stop=True)

            # tz = U/32 + db1, then transpose -> [f, b] (all pre-AR)
            tz_pad = wrk.tile([32, F], F32)
            nc.vector.memset(tz_pad[:], 0.0)
            nc.vector.tensor_add(tz_pad[0:B, 0:512], uf5[:], bcs(BC_DB1, 512))
            nc.vector.tensor_add(tz_pad[0:B, 512:F], uf2[:],
                                 bc_sb[0:B, BC_DB1 + 512:BC_DB1 + F])
            tzt = wrk.tile([128, FO * B], F32)
            for fo in range(FO):
                tp = pss.tile([128, 32], F32, name="ps", tag="ps")
                nc.tensor.transpose(tp[:], tz_pad[0:32, fo * 128:(fo + 1) * 128],
                                    eye)
                nc.vector.tensor_copy(tzt[:, fo * B:(fo + 1) * B], tp[:, 0:B])

            mh_bt = wrk.tile([32, HID], F32)
            nc.vector.memset(mh_bt[:], 0.0)
            osb = wrk.tile([112, F], F32)
            nc.vector.memset(osb[:], 0.0)
            # re-preload the activation LUTs close to the AR landing, ending
            # with Gelu so the tail's first act (G) hits a resident table
            scr3 = wrk.tile([1, 2], F32)
            nc.scalar.activation(scr3[:, 0:1], scr[:, 0:1], AF.Gelu_apprx_tanh)

            # ---- post-AR tail (f-major) ----
            # all activations split by readback half so the first chain
            # matmuls start one DMA-receipt latency earlier
            g_t = wrk.tile([128, FO * B], BF16)
            nc.scalar.activation(g_t[:, 0:3 * B], h1rt[:, 0:3 * B],
                                 AF.Gelu_apprx_tanh)
            nc.scalar.activation(g_t[:, 3 * B:FO * B], h1rt[:, 3 * B:FO * B],
                                 AF.Gelu_apprx_tanh)
            # gelu'(h) via central difference of the resident Gelu LUT:
            # (G(h+eps) - G(h-eps)) / (2 eps), eps=0.125; the 1/(2 eps)
            # factor is pre-folded into tz (via fs and db1)
            gpp = wrk.tile([128, FO * B], F32)
            gpm = wrk.tile([128, FO * B], F32)
            gpd = wrk.tile([128, FO * B], F32)
            z1_t = wrk.tile([128, FO * B], BF16)
            nc.scalar.activation(gpp[:, 0:3 * B], h1rt[:, 0:3 * B],
                                 AF.Gelu_apprx_tanh, bias=epsp[:])
            nc.scalar.activation(gpm[:, 0:3 * B], h1rt[:, 0:3 * B],
                                 AF.Gelu_apprx_tanh, bias=epsm[:])
            nc.vector.tensor_sub(gpd[:, 0:3 * B], gpp[:, 0:3 * B],
                                 gpm[:, 0:3 * B])
            nc.vector.tensor_mul(z1_t[:, 0:3 * B], tzt[:, 0:3 * B],
                                 gpd[:, 0:3 * B])
            nc.scalar.activation(gpp[:, 3 * B:FO * B], h1rt[:, 3 * B:FO * B],
                                 AF.Gelu_apprx_tanh, bias=epsp[:])
            nc.scalar.activation(gpm[:, 3 * B:FO * B], h1rt[:, 3 * B:FO * B],
                                 AF.Gelu_apprx_tanh, bias=epsm[:])
            nc.vector.tensor_sub(gpd[:, 3 * B:FO * B], gpp[:, 3 * B:FO * B],
                                 gpm[:, 3 * B:FO * B])
            nc.vector.tensor_mul(z1_t[:, 3 * B:FO * B], tzt[:, 3 * B:FO * B],
                                 gpd[:, 3 * B:FO * B])

            # four concurrent PE column-group chains; og/od/oz write into the
            # dead U PSUM rows, m1/cps own group 0:
            #   group (0,32): og = g @ W2    -> u rows 32:48
            #   group (0,64): od = g @ dW2   -> u rows 64:80
            #   group (0,96): oz = z1 @ W2   -> u rows 96:112
            og5, og2 = u5[32:32 + B, :], u2[32:32 + B, :]
            od5, od2 = u5[64:64 + B, :], u2[64:64 + B, :]
            oz5, oz2 = u5[96:96 + B, :], u2[96:96 + B, :]
            mps = pss.tile([B, HID], F32, name="ps", tag="ps")

            def mm_og(k):
                nc.tensor.matmul(og5, g_t[:, k * B:(k + 1) * B],
                                 w2_sb[:, k * F:k * F + 512],
                                 start=(k == 0), stop=(k == KF - 1),
                                 tile_position=(0, 32), skip_group_check=True)
                nc.tensor.matmul(og2, g_t[:, k * B:(k + 1) * B],
                                 w2_sb[:, k * F + 512:(k + 1) * F],
                                 start=(k == 0), stop=(k == KF - 1),
                                 tile_position=(0, 32), skip_group_check=True)

            def mm_od(k):
                nc.tensor.matmul(od5, g_t[:, k * B:(k + 1) * B],
                                 dw2_sb[:, k * F:k * F + 512],
                                 start=(k == 0), stop=(k == KF - 1),
                                 tile_position=(0, 64), skip_group_check=True)
                nc.tensor.matmul(od2, g_t[:, k * B:(k + 1) * B],
                                 dw2_sb[:, k * F + 512:(k + 1) * F],
                                 start=(k == 0), stop=(k == KF - 1),
                                 tile_position=(0, 64), skip_group_check=True)

            def mm_oz(k):
                nc.tensor.matmul(oz5, z1_t[:, k * B:(k + 1) * B],
                                 w2_sb[:, k * F:k * F + 512],
                                 start=(k == 0), stop=(k == KF - 1),
                                 tile_position=(0, 96), skip_group_check=True)
                nc.tensor.matmul(oz2, z1_t[:, k * B:(k + 1) * B],
                                 w2_sb[:, k * F + 512:(k + 1) * F],
                                 start=(k == 0), stop=(k == KF - 1),
                                 tile_position=(0, 96), skip_group_check=True)

            def mm_m1(k):
                nc.tensor.matmul(mps[:], g_t[:, k * B:(k + 1) * B],
                                 a1_sb[:, k * HID:(k + 1) * HID],
                                 start=(k == 0), stop=(k == KF - 1))

            # interleave so each group's next mm issues close behind its
            # predecessor; oz (z1-gated, ~1.7us later) enters mid-stream
            mm_m1(0); mm_og(0); mm_od(0)
            mm_m1(1); mm_og(1); mm_od(1)
            mm_m1(2); mm_m1(3); mm_og(2); mm_od(2)
            mm_m1(4); mm_m1(5); mm_og(3); mm_od(3)
            mtmp = wrk.tile([B, HID], F32)
            nc.vector.tensor_add(mtmp[:], mps[:], bcs(BC_A1V, HID))
            nc.vector.tensor_relu(mh_bt[0:B, :], mtmp[:])
            mm_oz(0); mm_og(4); mm_od(4)
            mm_oz(1); mm_og(5); mm_od(5)
            # meta transposes/cps interleave mid-chain so csel/CS land early
            mh_t = wrk.tile([128, 2 * B], BF16)
            tp = pss.tile([128, 32], F32, name="ps", tag="ps")
            nc.tensor.transpose(tp[:], mh_bt[0:32, 0:128], eye)
            nc.vector.tensor_copy(mh_t[:, 0:B], tp[:, 0:B])
            tp = pss.tile([128, 32], F32, name="ps", tag="ps")
            nc.tensor.transpose(tp[0:HID - 128, :], mh_bt[0:32, 128:HID], eye)
            nc.vector.tensor_copy(mh_t[0:HID - 128, B:2 * B],
                                  tp[0:HID - 128, 0:B])
            mm_oz(2)
            cps = pss.tile([B, T], F32, name="ps", tag="ps")
            nc.tensor.matmul(cps[:], mh_t[:, 0:B], mw2t_sb[:, 0:T],
                             start=True, stop=False)
            nc.tensor.matmul(cps[:], mh_t[0:HID - 128, B:2 * B],
                             mw2t_sb[0:HID - 128, T:2 * T],
                             start=False, stop=True)
            mm_oz(3); mm_oz(4); mm_oz(5)
            # og/od copies issue BEFORE the csel vector chain so they are
            # not queue-blocked behind it (their PSUM stops fire ~1.5us
            # before cps is even ready)
            nc.vector.tensor_copy(osb[0:80, 0:512], u5[0:80, :])
            nc.scalar.activation(osb[0:80, 512:F], u2[0:80, :], AF.Copy)
            nc.scalar.activation(osb[96:112, 0:512], u5[96:112, :], AF.Copy)
            nc.vector.tensor_copy(osb[96:112, 512:F], u2[96:112, :])
            coefs = wrk.tile([B, T], F32)
            nc.vector.tensor_add(coefs[:], cps[:], bcs(BC_MB2, T))
            cjunk = wrk.tile([B, T], F32)
            nc.vector.tensor_mul(cjunk[:], coefs[:], bcs(BC_SEL, T))
            csel = wrk.tile([B, 1], F32)
            nc.vector.reduce_sum(csel[:], cjunk[:], axis=mybir.AxisListType.X)
            nc.sync.dma_start(CS.ap(), csel[:])
            nc.sync.dma_start(OT.ap(), osb[32:112, :])

    nc.compile()
    return nc


def _get_nc():
    if "nc" not in _CACHE:
        _CACHE["nc"] = build()
    return _CACHE["nc"]


def _prep_in_maps(x, W1, b1, W2, b2, mW1, mb1, mW2, mb2, dW1, db1, dW2, db2):
    f32 = np.float32
    bf16 = ml_dtypes.bfloat16
    fp8 = ml_dtypes.float8_e4m3
    X = np.ascontiguousarray(np.asarray(x, f32).reshape(B, D))
    XT = np.ascontiguousarray(X.T)
    XTb = np.ascontiguousarray(
        XT.reshape(KD, 128, B).transpose(1, 0, 2).reshape(128, KD * B)
    ).astype(bf16)
    W1 = np.asarray(W1, f32)
    W2 = np.asarray(W2, f32)
    mW1 = np.asarray(mW1, f32)
    b1 = np.asarray(b1, f32); b2 = np.asarray(b2, f32)
    mb1 = np.asarray(mb1, f32); mb2 = np.asarray(mb2, f32)
    dW1 = np.asarray(dW1, f32); db1 = np.asarray(db1, f32)
    dW2 = np.asarray(dW2, f32); db2 = np.asarray(db2, f32)

    A1 = W2 @ mW1.T
    a1v = b2 @ mW1.T + mb1
    A1P = np.ascontiguousarray(
        A1.reshape(KF, 128, HID).transpose(1, 0, 2).reshape(128, KF * HID)
    ).astype(bf16)
    W2Pk = np.ascontiguousarray(
        W2.reshape(KF, 128, F).transpose(1, 0, 2).reshape(128, KF * F)
    ).astype(bf16)
    mw2t = np.ascontiguousarray(np.asarray(mW2, f32).T)
    MW2Tp = np.zeros((128, 2 * T), f32)
    MW2Tp[:, 0:T] = mw2t[0:128, :]
    MW2Tp[0:HID - 128, T:2 * T] = mw2t[128:HID, :]
    MW2Tp = MW2Tp.astype(bf16)

    # 4.0 = 1/(2*eps) for the central-difference gelu-derivative, folded
    # into the U fold and db1 so z1 = gp*(U+db1) comes out unscaled
    fsel = np.zeros((128, B), f32)
    for g in range(4):
        for m in range(B):
            fsel[32 * g + m, m] = 4.0 / FP8_SCALE

    in_maps = []
    for c in range(NCORES):
        bc = np.zeros((32, BC_W), f32)
        bc[0:B, BC_B1:BC_B1 + F] = b1[None, :]
        bc[0:B, BC_DB1:BC_DB1 + F] = 4.0 * db1[c][None, :]
        bc[0:B, BC_A1V:BC_A1V + HID] = a1v[None, :]
        bc[0:B, BC_MB2:BC_MB2 + T] = mb2[None, :]
        bc[0:B, BC_SEL + c] = 1.0
        bc[0:B, BC_ISB] = 1.0 if c == 0 else 0.0
        bc[0:32, BC_EYE:BC_EYE + 32] = np.eye(32, dtype=f32)
        in_maps.append({
            "xts": np.ascontiguousarray(XTb[:, c * KSH * B:(c + 1) * KSH * B]),
            "xt": XTb,
            "w1s": np.ascontiguousarray(
                W1[c * DSH:(c + 1) * DSH, :].reshape(KSH, 128, F)
                .transpose(1, 0, 2).reshape(128, KSH * F)).astype(bf16),
            "dw1": np.ascontiguousarray(
                (dW1[c] * FP8_SCALE).reshape(KD, 128, F)
                .transpose(1, 0, 2).reshape(128, KD * F)).astype(fp8),
            "w2p": W2Pk,
            "dw2p": np.ascontiguousarray(
                dW2[c].reshape(KF, 128, F).transpose(1, 0, 2)
                .reshape(128, KF * F)).astype(bf16),
            "a1p": A1P,
            "mw2t": MW2Tp,
            "bc": bc,
            "fs": fsel,
        })
    return in_maps


def run(inputs, trace=False, trace_cores=None, tmpdir=None):
    nc = _get_nc()
    in_maps = _prep_in_maps(**inputs)
    res = bass_utils.run_bass_kernel_spmd(
        nc, in_maps, core_ids=list(range(NCORES)), trace=trace,
        trace_cores=trace_cores, tmpdir=tmpdir)
    db2 = np.asarray(inputs["db2"], np.float64)
    b2 = np.asarray(inputs["b2"], np.float64)
    acc = np.zeros((B, F), np.float64)
    acc += b2[None, :]
    for c in range(NCORES):
        ot = res.results[c]["ot"].astype(np.float64)
        cs = res.results[c]["cs"].astype(np.float64)      # [B, 1]
        # rows 0:16 = g@W2 (core 0 only), 32:48 = g@dW2 (x csel),
        # 64:80 = z1@W2 (x csel)
        acc += cs * (ot[32:32 + B] + ot[64:64 + B]) + cs * db2[c][None, :]
        if c == 0:
            acc += ot[0:B]
    return acc.astype(np.float32), res


def kernel(**inputs):
    out, _ = run(inputs, trace=False)
    return out



# revision 3
# speedup vs baseline: 1.6961x; 1.6961x over previous
"""Trainium2 Bass kernel for nn_MetaNetLinearizedModel (v3: no-collective
F-sharding).

Each core owns a 96-column slice fc of the feature dim F=768 and computes,
fully locally (no AllReduce):
    z1_c  = X @ W1[:, fc]                      (bf16, f32 accum)
    g_c   = gelu(z1_c + b1[fc]) ;  gp_c = gelu'(...) via central difference
    U_t,c = X @ dW1[t][:, fc]                  (fp8 moving, all 8 tasks)
    v_t,c = gp_c * (U_t,c + db1[t][fc])
    P_t,c = v_t,c @ W2[fc, :] + g_c @ dW2[t][fc, :]    -> PO rows 16t:16t+16
    fo_c  = g_c @ W2[fc, :]                            -> FO (feats partial)
The host sums feats partials across cores, runs the tiny meta-net for
coefs, and forms
    out = feats + b2 + sum_t coefs[:,t] * P_t + coefs @ db2.
The dW1 stream is split k-major into two halves (tasks 0-3 on the sync
ring, tasks 4-7 on the scalar ring); each half's reduce/v-term tail can
drain while the other ring still streams.  Outputs leave via the idle
gpsimd queue.  No collective, no cs export, tail is ~2 matmul pairs.
"""
import sys

sys.path.insert(0, "/opt/trn_rl_repo")

import numpy as np
import ml_dtypes
import concourse.bass as bass
import concourse.bacc as bacc
import concourse.tile as tile
import concourse.mybir as mybir
from concourse import bass_utils

F32 = mybir.dt.float32
BF16 = mybir.dt.bfloat16
FP8 = mybir.dt.float8e4
AF = mybir.ActivationFunctionType
OP = mybir.AluOpType

B = 16
D = 3 * 64 * 64        # 12288
F = 768
HID = 192
T = 8
NCORES = 8
FSH = F // NCORES      # 96 columns of F per core
KD = D // 128          # 96 k-tiles
FP8_SCALE = 32.0
EPS = 0.125            # central-difference step for gelu'

HW = 4 * FSH           # 384 dW1 columns per k-tile per half (4 tasks)
NW1C = 8               # w1 slice chunks (12 k-tiles each)
NDWC = 12              # dW1 chunks per half (8 k-tiles each)
W1CK = KD // NW1C      # 12
DWCK = KD // NDWC      # 8

_CACHE = {}


def build():
    nc = bacc.Bacc("TRN2", target_bir_lowering=False, debug=False,
                   enable_asserts=False, num_devices=NCORES)

    XT = nc.dram_tensor("xt", [128, KD * B], BF16, kind="ExternalInput")
    W1S = nc.dram_tensor("w1s", [128, KD * FSH], BF16, kind="ExternalInput")
    DW1A = nc.dram_tensor("dw1a", [128, KD * HW], FP8, kind="ExternalInput")
    DW1B = nc.dram_tensor("dw1b", [128, KD * HW], FP8, kind="ExternalInput")
    W2P = nc.dram_tensor("w2p", [FSH, F], BF16, kind="ExternalInput")
    DW2P = nc.dram_tensor("dw2p", [FSH, T * F], BF16, kind="ExternalInput")
    FSB = nc.dram_tensor("fsb", [128, 32], BF16, kind="ExternalInput")
    CONS = nc.dram_tensor("cons", [128, 35], F32, kind="ExternalInput")
    DB1R = nc.dram_tensor("db1r", [B, F], F32, kind="ExternalInput")
    PO = nc.dram_tensor("po", [128, F], BF16, kind="ExternalOutput")
    FO = nc.dram_tensor("fo", [B, F], F32, kind="ExternalOutput")

    with tile.TileContext(nc, num_cores=NCORES) as tc:
        with (
            tc.tile_pool(name="cst", bufs=1) as cst,
            tc.tile_pool(name="w1c", bufs=NW1C) as w1cp,
            tc.tile_pool(name="dwa", bufs=NDWC) as dwap,
            tc.tile_pool(name="dwb", bufs=NDWC) as dwbp,
            tc.tile_pool(name="wrk", bufs=1) as wrk,
            tc.tile_pool(name="psa", bufs=1, space="PSUM") as psa,
            tc.tile_pool(name="psb", bufs=1, space="PSUM") as psb,
            tc.tile_pool(name="psu", bufs=1, space="PSUM") as psu,
            tc.tile_pool(name="pss", bufs=2, space="PSUM") as pss,
        ):
            # ---- activation LUT preload (gelu table resident early) ----
            scr = wrk.tile([1, 2], F32)
            nc.vector.memset(scr[:], 0.0)
            scr2 = wrk.tile([1, 2], F32)
            nc.scalar.activation(scr2[:, 0:1], scr[:, 0:1], AF.Gelu_apprx_tanh)

            # ---- DMA kicks ----
            # sync ring:   xt, w1s c0-c5, dW1A chunks          (~6.9 MB)
            # scalar ring: consts, w1s c6-c7, w2/dw2, dW1B     (~6.7 MB)
            xt_sb = cst.tile([128, KD * B], BF16)
            nc.sync.dma_start(xt_sb[:], XT.ap())
            w1c = []
            for j in range(NW1C):
                wt = w1cp.tile([128, W1CK * FSH], BF16, name="w1t", tag="w1t")
                w1c.append(wt)
            for j in range(6):
                nc.sync.dma_start(
                    w1c[j][:],
                    W1S.ap()[:, j * W1CK * FSH:(j + 1) * W1CK * FSH])
            dwa = []
            for j in range(NDWC):
                t_ = dwap.tile([128, DWCK * HW], FP8, name="dwat", tag="dwat")
                nc.sync.dma_start(
                    t_[:], DW1A.ap()[:, j * DWCK * HW:(j + 1) * DWCK * HW])
                dwa.append(t_)

            fsb_sb = cst.tile([128, 32], BF16)
            nc.scalar.dma_start(fsb_sb[:], FSB.ap())
            cons_sb = cst.tile([128, 35], F32)
            nc.scalar.dma_start(cons_sb[:], CONS.ap())
            db1r_sb = cst.tile([B, F], F32)
            nc.scalar.dma_start(db1r_sb[:], DB1R.ap())
            for j in range(6, NW1C):
                nc.scalar.dma_start(
                    w1c[j][:],
                    W1S.ap()[:, j * W1CK * FSH:(j + 1) * W1CK * FSH])
            w2_sb = cst.tile([FSH, F], BF16)
            nc.scalar.dma_start(w2_sb[:], W2P.ap())
            dw2_sb = cst.tile([FSH, T * F], BF16)
            nc.scalar.dma_start(dw2_sb[:], DW2P.ap())
            dwb = []
            for j in range(NDWC):
                t_ = dwbp.tile([128, DWCK * HW], FP8, name="dwbt", tag="dwbt")
                nc.scalar.dma_start(
                    t_[:], DW1B.ap()[:, j * DWCK * HW:(j + 1) * DWCK * HW])
                dwb.append(t_)

            eye = cons_sb[0:32, 0:32]
            b1c = cons_sb[0:FSH, 32:33]
            b1p = cons_sb[0:FSH, 33:34]
            b1m = cons_sb[0:FSH, 34:35]
            fsA = fsb_sb[:, 0:16]
            fsB = fsb_sb[:, 16:32]

            # ---- z1 = X @ W1[:, fc]  (single accumulation group) ----
            z1ps = pss.tile([B, FSH], F32, name="sp", tag="sp",
                            padded_shape=[128, 512])
            for k in range(KD):
                wt = w1c[k // W1CK]
                q = (k % W1CK) * FSH
                nc.tensor.matmul(z1ps[:], xt_sb[:, k * B:(k + 1) * B],
                                 wt[:, q:q + FSH],
                                 start=(k == 0), stop=(k == KD - 1),
                                 skip_group_check=True)

            # z1 -> f-major [f, b] via PE transpose
            z1pad = wrk.tile([32, FSH], F32)
            nc.vector.tensor_copy(z1pad[0:B, :], z1ps[:])
            z1tp = pss.tile([FSH, 32], F32, name="sp", tag="sp",
                            padded_shape=[128, 512])
            nc.tensor.matmul(z1tp[:], z1pad[:], eye, is_transpose=True,
                             skip_group_check=True)
            z1t = wrk.tile([FSH, B], F32)
            nc.vector.tensor_copy(z1t[:], z1tp[:, 0:B])

            # g = gelu(z1+b1); gpd = G(z1+b1+eps) - G(z1+b1-eps)
            # (the 1/(2 eps) = 4.0 factor is folded into fsA/fsB and db1r)
            gT = wrk.tile([FSH, B], BF16)
            nc.scalar.activation(gT[:], z1t[:], AF.Gelu_apprx_tanh, bias=b1c)
            gpp = wrk.tile([FSH, B], F32)
            nc.scalar.activation(gpp[:], z1t[:], AF.Gelu_apprx_tanh, bias=b1p)
            gpm = wrk.tile([FSH, B], F32)
            nc.scalar.activation(gpm[:], z1t[:], AF.Gelu_apprx_tanh, bias=b1m)
            gpd = wrk.tile([FSH, B], F32)
            nc.vector.tensor_sub(gpd[:], gpp[:], gpm[:])

            # pair stationaries for the g @ dW2 term: [g|0] and [0|g]
            gpe = wrk.tile([FSH, 32], BF16)
            nc.vector.memset(gpe[:], 0.0)
            gpo = wrk.tile([FSH, 32], BF16)
            nc.vector.memset(gpo[:], 0.0)
            nc.vector.tensor_copy(gpe[:, 0:16], gT[:])
            nc.vector.tensor_copy(gpo[:, 16:32], gT[:])

            # ---- feats partial: fo = g @ W2[fc, :] ----
            fps5 = pss.tile([B, 512], F32, name="sp", tag="sp",
                            padded_shape=[128, 512])
            nc.tensor.matmul(fps5[:], gT[:], w2_sb[:, 0:512],
                             start=True, stop=True, skip_group_check=True)
            fps2 = pss.tile([B, 256], F32, name="sp", tag="sp",
                            padded_shape=[128, 512])
            nc.tensor.matmul(fps2[:], gT[:], w2_sb[:, 512:F],
                             start=True, stop=True, skip_group_check=True)
            fo_sb = wrk.tile([B, F], F32)
            nc.vector.tensor_copy(fo_sb[:, 0:512], fps5[:])
            nc.vector.tensor_copy(fo_sb[:, 512:F], fps2[:])
            nc.gpsimd.dma_start(FO.ap(), fo_sb[:])

            # ---- P accumulation: pair group j owns rows 32j:32j+32 ----
            P5 = psu.tile([128, 512], F32, name="p5")
            P2 = psu.tile([128, 256], F32, name="p2",
                          padded_shape=[128, 512])
            ufps = psu.tile([128, HW], F32, name="uf",
                            padded_shape=[128, 512])
            vps = wrk.tile([FSH, 128], BF16)

            def mm_gterm(t):
                j = t // 2
                st = gpe if t % 2 == 0 else gpo
                nc.tensor.matmul(P5[32 * j:32 * j + 32, :], st[:],
                                 dw2_sb[:, t * F:t * F + 512],
                                 start=(t % 2 == 0), stop=False,
                                 tile_position=(0, 32 * j),
                                 skip_group_check=True)
                nc.tensor.matmul(P2[32 * j:32 * j + 32, :], st[:],
                                 dw2_sb[:, t * F + 512:(t + 1) * F],
                                 start=(t % 2 == 0), stop=False,
                                 tile_position=(0, 32 * j),
                                 skip_group_check=True)

            # ---- U streams: A = tasks 0-3 (sync), B = tasks 4-7 (scalar)
            # A accumulation groups at PE cols 0/64 -> psum rows 0:16, 64:80
            # B accumulation groups at PE cols 32/96 -> rows 32:48, 96:112
            upsA = psa.tile([128, HW], F32, name="ua",
                            padded_shape=[128, 512])
            upsB = psb.tile([128, HW], F32, name="ub",
                            padded_shape=[128, 512])

            def mm_u(ups, chunks, k, rowoffs):
                src = chunks[k // DWCK]
                q = (k % DWCK) * HW
                ro = rowoffs[k % 2]
                nc.tensor.matmul(ups[ro:ro + B, :],
                                 xt_sb[:, k * B:(k + 1) * B],
                                 src[:, q:q + HW],
                                 start=(k < 2), stop=(k >= KD - 2),
                                 tile_position=(0, ro),
                                 skip_group_check=True)

            def chunkA(j):
                for k in range(j * DWCK, (j + 1) * DWCK):
                    mm_u(upsA, dwa, k, (0, 64))

            def chunkB(j):
                for k in range(j * DWCK, (j + 1) * DWCK):
                    mm_u(upsB, dwb, k, (32, 96))

            def drain_half(ups, fsv, ufrow, tbase):
                usb = wrk.tile([128, HW], BF16, name="usb", tag="usb")
                nc.vector.tensor_copy(usb[:], ups[:])
                nc.tensor.matmul(ufps[ufrow:ufrow + B, :], fsv, usb[:],
                                 start=True, stop=True,
                                 tile_position=(0, ufrow),
                                 skip_group_check=True)
                for tt in range(4):
                    t = tbase + tt
                    tzpad = wrk.tile([32, FSH], F32, name="tz", tag="tz",
                                     bufs=2)
                    nc.vector.tensor_add(
                        tzpad[0:B, :],
                        ufps[ufrow:ufrow + B, tt * FSH:(tt + 1) * FSH],
                        db1r_sb[:, t * FSH:(t + 1) * FSH])
                    tztp = pss.tile([FSH, 32], F32, name="sp", tag="sp",
                                    padded_shape=[128, 512])
                    nc.tensor.matmul(tztp[:], tzpad[:], eye,
                                     is_transpose=True,
                                     skip_group_check=True)
                    nc.vector.tensor_mul(vps[:, t * B:(t + 1) * B],
                                         tztp[:, 0:B], gpd[:])
                for j in (tbase // 2, tbase // 2 + 1):
                    nc.tensor.matmul(P5[32 * j:32 * j + 32, :],
                                     vps[:, 32 * j:32 * j + 32],
                                     w2_sb[:, 0:512],
                                     start=False, stop=True,
                                     tile_position=(0, 32 * j),
                                     skip_group_check=True)
                    nc.tensor.matmul(P2[32 * j:32 * j + 32, :],
                                     vps[:, 32 * j:32 * j + 32],
                                     w2_sb[:, 512:F],
                                     start=False, stop=True,
                                     tile_position=(0, 32 * j),
                                     skip_group_check=True)

            # PE emission order tracks expected data-arrival order
            chunkB(0)
            for t in range(T):
                mm_gterm(t)
            chunkB(1)
            chunkB(2)
            for j in range(8):
                chunkA(j)
                chunkB(j + 3)
            chunkA(8)
            chunkA(9)
            chunkA(10)
            chunkB(11)
            drain_half(upsB, fsB, 32, 4)
            chunkA(11)
            drain_half(upsA, fsA, 0, 0)

            # ---- export P ----
            po_sb = wrk.tile([128, F], BF16)
            nc.vector.tensor_copy(po_sb[:, 0:512], P5[:])
            nc.vector.tensor_copy(po_sb[:, 512:F], P2[:])
            nc.gpsimd.dma_start(PO.ap(), po_sb[:])

    nc.compile()
    return nc


def _get_nc():
    if "nc" not in _CACHE:
        _CACHE["nc"] = build()
    return _CACHE["nc"]


def _prep_in_maps(x, W1, b1, W2, b2, mW1, mb1, mW2, mb2, dW1, db1, dW2, db2):
    f32 = np.float32
    bf16 = ml_dtypes.bfloat16
    fp8 = ml_dtypes.float8_e4m3
    X = np.ascontiguousarray(np.asarray(x, f32).reshape(B, D))
    XT = np.ascontiguousarray(X.T)
    XTb = np.ascontiguousarray(
        XT.reshape(KD, 128, B).transpose(1, 0, 2).reshape(128, KD * B)
    ).astype(bf16)
    W1 = np.asarray(W1, f32)
    W2 = np.asarray(W2, f32)
    b1 = np.asarray(b1, f32)
    dW1 = np.asarray(dW1, f32)
    db1 = np.asarray(db1, f32)
    dW2 = np.asarray(dW2, f32)

    # fsA rows {m, 64+m}, fsB rows {32+m, 96+m}; the value 4/32 folds the
    # 1/(2 eps) gelu'-difference factor and the fp8 scale
    fsv = np.zeros((128, 32), f32)
    for m in range(B):
        fsv[m, m] = 4.0 / FP8_SCALE
        fsv[64 + m, m] = 4.0 / FP8_SCALE
        fsv[32 + m, 16 + m] = 4.0 / FP8_SCALE
        fsv[96 + m, 16 + m] = 4.0 / FP8_SCALE
    FSBv = fsv.astype(bf16)

    in_maps = []
    for c in range(NCORES):
        fc = slice(c * FSH, (c + 1) * FSH)
        w1s = np.ascontiguousarray(
            W1[:, fc].reshape(KD, 128, FSH).transpose(1, 0, 2)
            .reshape(128, KD * FSH)).astype(bf16)
        dwa = np.ascontiguousarray(
            (dW1[0:4, :, fc] * FP8_SCALE).transpose(1, 0, 2)
            .reshape(KD, 128, HW).transpose(1, 0, 2)
            .reshape(128, KD * HW)).astype(fp8)
        dwb = np.ascontiguousarray(
            (dW1[4:8, :, fc] * FP8_SCALE).transpose(1, 0, 2)
            .reshape(KD, 128, HW).transpose(1, 0, 2)
            .reshape(128, KD * HW)).astype(fp8)
        w2p = np.ascontiguousarray(W2[fc, :]).astype(bf16)
        dw2p = np.ascontiguousarray(
            dW2[:, fc, :].transpose(1, 0, 2).reshape(FSH, T * F)).astype(bf16)
        cons = np.zeros((128, 35), f32)
        cons[0:32, 0:32] = np.eye(32, dtype=f32)
        cons[0:FSH, 32] = b1[fc]
        cons[0:FSH, 33] = b1[fc] + EPS
        cons[0:FSH, 34] = b1[fc] - EPS
        db1r = np.ascontiguousarray(np.broadcast_to(
            (4.0 * db1[:, fc]).reshape(T * FSH), (B, F))).astype(f32)
        in_maps.append({
            "xt": XTb,
            "w1s": w1s,
            "dw1a": dwa,
            "dw1b": dwb,
            "w2p": w2p,
            "dw2p": dw2p,
            "fsb": FSBv,
            "cons": cons,
            "db1r": db1r,
        })
    return in_maps


def run(inputs, trace=False, trace_cores=None, tmpdir=None):
    nc = _get_nc()
    in_maps = _prep_in_maps(**inputs)
    res = bass_utils.run_bass_kernel_spmd(
        nc, in_maps, core_ids=list(range(NCORES)), trace=trace,
        trace_cores=trace_cores, tmpdir=tmpdir)

    f64 = np.float64
    b2 = np.asarray(inputs["b2"], f64)
    mW1 = np.asarray(inputs["mW1"], f64)
    mb1 = np.asarray(inputs["mb1"], f64)
    mW2 = np.asarray(inputs["mW2"], f64)
    mb2 = np.asarray(inputs["mb2"], f64)
    db2 = np.asarray(inputs["db2"], f64)

    feats = np.zeros((B, F), f64)
    P = np.zeros((128, F), f64)
    for c in range(NCORES):
        feats += res.results[c]["fo"].astype(f64)
        P += res.results[c]["po"].astype(f64)
    feats += b2[None, :]
    h = np.maximum(feats @ mW1.T + mb1, 0.0)
    coefs = h @ mW2.T + mb2                     # [B, T]
    out = feats + coefs @ db2
    for t in range(T):
        out += coefs[:, t:t + 1] * P[t * B:(t + 1) * B]
    return out.astype(np.float32), res


def kernel(**inputs):
    out, _ = run(inputs, trace=False)
    return out
